# revision 57
# baseline (speedup 1.0000x reference)
"""Trainium2 Bass kernel for the Gudi UpProj block.

Reference computation (per image, NCHW):
    xu  = zero_stuff_2x(x)                    # [B,1024,32,32], nonzero only at even (h,w)
    c1  = conv5x5(xu, w1, pad=2);  out1 = relu(BN(c1))
    c2  = conv3x3(out1, w2, pad=1)
    csc = conv5x5(xu, wsc, pad=2)
    out = relu(BN(c2) + BN(csc))              # BN: training-mode batch stats over (N,H,W)

Strategy:
  * Data-parallel over batch: 16 images -> 2 per NeuronCore (8 cores).
  * Zero-stuffing exploited: a 5x5 conv on the zero-stuffed 32x32 grid decomposes
    into 4 parity phases, each a small conv (3x3 / 3x2 / 2x3 / 2x2) on the original
    16x16 grid -> 4x FLOP reduction.
  * All convs as tap-decomposed matmuls on the PE array in float32r
    (TF32-like: full PE rate at N>=256, ~1e-4 rel err). Weights / x are
    pre-rounded host-side (RNE to 11 mantissa bits), regrouped into the exact
    consumption order, and DMA'd directly in multi-tap batches.
  * BN batch stats (sum, sumsq per channel) need cross-core reduction:
    two small AllReduces (stats of c1; stats of c2+csc together).
"""

import numpy as np

NCORES = 8
B = 16
B_LOC = B // NCORES          # 2 images per core
CIN, COUT = 1024, 512
NCI, NCO = CIN // 128, COUT // 128   # 8, 4 partition tiles
H = 16                        # input spatial
OH = 32                       # output spatial
EPS = 1e-5
CNT = float(B * OH * OH)      # BN element count per channel = 16384
PHASES = [(0, 0), (0, 1), (1, 0), (1, 1)]

_CACHE = {}


def _to_bf16(a: np.ndarray) -> np.ndarray:
    """Round fp32 to bfloat16 (RNE) - matmul operand dtype on the PE."""
    import ml_dtypes
    return np.ascontiguousarray(a, dtype=np.float32).astype(ml_dtypes.bfloat16)


def _taps(p):
    """Taps of a parity phase along one dim: list of (input shift, 5-tap kernel idx)."""
    if p == 0:
        return [(-1, 0), (0, 2), (1, 4)]
    return [(0, 1), (1, 3)]


def _w5_groups():
    """Weight-block groups for the phase-decomposed 5x5 conv, in consumption
    order: one group per (phase, cin-tile, kernel-row) holding len(kws) blocks."""
    groups = []
    for (p, q) in PHASES:
        for ci in range(NCI):
            for (ah, kh) in _taps(p):
                groups.append((p, q, ci, ah, kh, _taps(q)))
    return groups


def _phase_view(ap2048, p, q):
    """[128, 2048] tile viewed as [128, b, i, j] at output positions (2i+p, 2j+q)."""
    v = ap2048.rearrange("c (b i p2 j q2) -> c b i p2 j q2", b=2, i=16, p2=2, j=16, q2=2)
    return v[:, :, :, p, :, q]


def _build_nc():
    import concourse.mybir as mybir
    import concourse.tile as tile
    from concourse import bacc

    f32 = mybir.dt.float32
    bf16 = mybir.dt.bfloat16
    ALU = mybir.AluOpType
    AFT = mybir.ActivationFunctionType

    nc = bacc.Bacc("TRN2", target_bir_lowering=False, debug=False)

    # ---- kernel I/O ----
    xpad_d = nc.dram_tensor("xpad", [B_LOC, CIN, 18, 18], bf16, kind="ExternalInput").ap()
    w1g_d = nc.dram_tensor("w1g", [200, 128, COUT], bf16, kind="ExternalInput").ap()
    wscg_d = nc.dram_tensor("wscg", [200, 128, COUT], bf16, kind="ExternalInput").ap()
    w2t_d = nc.dram_tensor("w2t", [NCO, NCO, 128, 9, 128], bf16, kind="ExternalInput").ap()
    gb_d = nc.dram_tensor("gb", [128, 6, 4], f32, kind="ExternalInput").ap()
    zpad_d = nc.dram_tensor("zpad", [2, 34, 34], bf16, kind="ExternalInput").ap()
    out_d = nc.dram_tensor("out", [B_LOC, COUT, OH, OH], f32, kind="ExternalOutput").ap()

    with tile.TileContext(nc) as tc:
        # collective buffers (internal DRAM)
        _frees = []
        ar1_in, _f = tc.tile([2, COUT], f32, space="DRAM", name="ar1_in"); _frees.append(_f)
        ar1_out, _f = tc.tile([2, COUT], f32, space="DRAM", addr_space="Shared", name="ar1_out"); _frees.append(_f)
        arsc_in, _f = tc.tile([2, COUT], f32, space="DRAM", name="arsc_in"); _frees.append(_f)
        arsc_out, _f = tc.tile([NCORES, 2, COUT], f32, space="DRAM", addr_space="Shared", name="arsc_out"); _frees.append(_f)
        arA_in, _f = tc.tile([2, 384], f32, space="DRAM", name="arA_in"); _frees.append(_f)
        arA_out, _f = tc.tile([NCORES, 2, 384], f32, space="DRAM", addr_space="Shared", name="arA_out"); _frees.append(_f)
        arB_in, _f = tc.tile([2, 128], f32, space="DRAM", name="arB_in"); _frees.append(_f)
        arB_out, _f = tc.tile([NCORES, 2, 128], f32, space="DRAM", addr_space="Shared", name="arB_out"); _frees.append(_f)

        with tc.tile_pool(name="xp", bufs=1) as xp_pool, \
             tc.tile_pool(name="acts", bufs=1) as acts, \
             tc.tile_pool(name="op1", bufs=1) as op1_pool, \
             tc.tile_pool(name="wts", bufs=4) as wts, \
             tc.tile_pool(name="w2p", bufs=8) as w2p, \
             tc.tile_pool(name="scr", bufs=1) as scr_pool, \
             tc.tile_pool(name="small", bufs=1) as small, \
             tc.tile_pool(name="ps", bufs=8, space="PSUM") as ps:

            # ---- persistent SBUF tensors ----
            XP = [xp_pool.tile([128, 2, 18, 18], bf16, name=f"xp{i}", tag=f"xp{i}")
                  for i in range(NCI)]
            C1 = [acts.tile([128, 2048], f32, name=f"c1_{i}", tag=f"c1_{i}") for i in range(NCO)]
            CSC = [acts.tile([128, 2048], bf16, name=f"csc_{i}", tag=f"csc_{i}") for i in range(NCO)]
            C2 = [acts.tile([128, 2048], bf16, name=f"c2_{i}", tag=f"c2_{i}") for i in range(NCO)]
            OP1 = [op1_pool.tile([128, 2, 34, 34], bf16, name=f"op1_{i}", tag=f"op1_{i}")
                   for i in range(NCO)]

            # stat columns: sums/sumsqs per (tensor, co, phase-or-quarter)
            sums1 = small.tile([128, 16], f32, name="sums1")
            sq1 = small.tile([128, 16], f32, name="sq1")
            sums2 = small.tile([128, 16], f32, name="sums2")
            sq2 = small.tile([128, 16], f32, name="sq2")
            sumssc = small.tile([128, 16], f32, name="sumssc")
            sqsc = small.tile([128, 16], f32, name="sqsc")
            pack1 = small.tile([128, 2, 4], f32, name="pack1")
            st1 = small.tile([128, 2, 4], f32, name="st1")
            gbv = small.tile([128, 6, 4], f32, name="gbv")      # rows: g1,b1,g2,b2,gsc,bsc
            scale1 = small.tile([128, 4], f32, name="scale1")
            shift1 = small.tile([128, 4], f32, name="shift1")
            packsc = small.tile([128, 2, 4], f32, name="packsc")
            stgsc = small.tile([128, NCORES, 2, 4], f32, name="stgsc")
            stsc = small.tile([128, 2, 4], f32, name="stsc")
            scalesc = small.tile([128, 4], f32, name="scalesc")
            shiftsc = small.tile([128, 4], f32, name="shiftsc")
            packA = small.tile([128, 2, 3], f32, name="packA")
            stgA = small.tile([128, NCORES, 2, 3], f32, name="stgA")
            stA = small.tile([128, 2, 3], f32, name="stA")
            scale2A = small.tile([128, 3], f32, name="scale2A")
            shift2A = small.tile([128, 3], f32, name="shift2A")
            packB = small.tile([128, 2, 1], f32, name="packB")
            stgB = small.tile([128, NCORES, 2], f32, name="stgB")
            stB = small.tile([128, 2, 1], f32, name="stB")
            scale2B = small.tile([128, 1], f32, name="scale2B")
            shift2B = small.tile([128, 1], f32, name="shift2B")
            tmpa = small.tile([128, 4], f32, name="tmpa")
            tmpb = small.tile([128, 4], f32, name="tmpb")
            epsc = small.tile([128, 1], f32, name="epsc")
            FSTG = [small.tile([128, 2048], f32, name=f"fstg{i}") for i in range(2)]

            # ---- input DMAs (x first: the PE's first dependency) ----
            def emit_xp_dma(ci):
                nc.sync.dma_start(
                    XP[ci][:].rearrange("c b h w -> c b (h w)"),
                    xpad_d[:, ci * 128:(ci + 1) * 128].rearrange("b c h w -> c b (h w)"),
                )
            emit_xp_dma(0)
            nc.vector.memset(epsc[:], EPS)

            # ---- PE warmup: dummy matmuls on a memset tile while the first
            # x/weight DMAs are in flight, so the PE p-state is fully ramped
            # (>3us continuous busy) when real work arrives ----
            warm = small.tile([128, 512], bf16, name="warm")
            nc.vector.memset(warm[:], 0.0)
            wps = ps.tile([128, 512], f32, name="warmps", tag="psb")
            NWARM = 14
            for wi in range(NWARM):
                nc.tensor.matmul(wps[:, :256], warm[:, :128], warm[:, :256],
                                 start=(wi == 0), stop=(wi == NWARM - 1))

            # ---- helper: one 5x5-phase-decomposed conv (conv1 / convsc) ----
            def conv5(wg_d, dst, sums, sqs, wtag, prefetch_xp=False):
                gofs = 0
                for iph, (p, q) in enumerate(PHASES):
                    pps = [ps.tile([128, 512], f32, name=f"{wtag}ps{iph}_{co}", tag="psb")
                           for co in range(NCO)]
                    kws = _taps(q)
                    n_acc = NCI * len(_taps(p)) * len(kws)
                    k = 0
                    for ci in range(NCI):
                        for ti, (ah, kh) in enumerate(_taps(p)):
                            L = len(kws)
                            wt = wts.tile([128, 3, 512], bf16, name=f"{wtag}w", tag="w5")
                            if prefetch_xp and gofs == 0:
                                # per-tap DMAs so the very first matmul's
                                # weights land sooner
                                for kwi in range(L):
                                    nc.sync.dma_start(
                                        wt[:, kwi:kwi + 1, :],
                                        wg_d[kwi:kwi + 1].rearrange("l c m -> c l m"))
                            else:
                                nc.sync.dma_start(
                                    wt[:, :L, :],
                                    wg_d[gofs:gofs + L].rearrange("l c m -> c l m"))
                            gofs += L
                            # x-tile prefetch behind the first weight group so
                            # the first matmul's dependencies DMA first
                            if prefetch_xp and iph == 0 and ti == 0:
                                if ci == 0:
                                    emit_xp_dma(1)
                                if ci + 2 < NCI:
                                    emit_xp_dma(ci + 2)
                            for kwi, (aw, kw) in enumerate(kws):
                                rhs = XP[ci][:, :, 1 + ah:17 + ah, 1 + aw:17 + aw]
                                for co in range(NCO):
                                    nc.tensor.matmul(
                                        pps[co][:], wt[:, kwi, co * 128:(co + 1) * 128], rhs,
                                        start=(k == 0), stop=(k == n_acc - 1))
                                k += 1
                    for co in range(NCO):
                        icol = co * 4 + iph
                        nc.vector.tensor_scalar(
                            dst[co][:, iph * 512:(iph + 1) * 512], pps[co][:],
                            0.0, 0.0, op0=ALU.add, op1=ALU.add,
                            accum_out=sums[:, icol:icol + 1])
                        scr = scr_pool.tile([128, 512], f32, name=f"{wtag}scr", tag="scr")
                        nc.scalar.activation(
                            scr[:], pps[co][:], AFT.Square,
                            accum_out=sqs[:, icol:icol + 1])

            # ================= conv1 =================
            conv5(w1g_d, C1, sums1, sq1, "c1", prefetch_xp=True)

            # aux DMAs (needed from BN1-apply onward; emitted late to keep the
            # startup DMA path clear)
            nc.sync.dma_start(gbv[:], gb_d)
            for co in range(NCO):
                nc.sync.dma_start(OP1[co][:], zpad_d.unsqueeze(0).partition_broadcast(128))

            # ---- c1 stats -> AllReduce #1 (overlaps with convsc compute) ----
            nc.vector.tensor_reduce(
                pack1[:, 0, :], sums1[:].rearrange("c (co ph) -> c co ph", ph=4),
                axis=mybir.AxisListType.X, op=ALU.add)
            nc.vector.tensor_reduce(
                pack1[:, 1, :], sq1[:].rearrange("c (co ph) -> c co ph", ph=4),
                axis=mybir.AxisListType.X, op=ALU.add)
            nc.sync.dma_start(ar1_in[:].rearrange("s (co c) -> c s co", c=128), pack1[:])
            nc.gpsimd.collective_compute(
                "AllReduce", ALU.add,
                replica_groups=[list(range(NCORES))],
                ins=[ar1_in.opt()], outs=[ar1_out.opt()])
            nc.sync.dma_start(st1[:], ar1_out[:].rearrange("s (co c) -> c s co", c=128))

            # ================= convsc (independent of BN1) =================
            conv5(wscg_d, CSC, sumssc, sqsc, "sc")

            # ---- csc stats -> AllGather #sc (overlaps BN1-apply + conv2) ----
            nc.vector.tensor_reduce(
                packsc[:, 0, :], sumssc[:].rearrange("c (co x) -> c co x", x=4),
                axis=mybir.AxisListType.X, op=ALU.add)
            nc.vector.tensor_reduce(
                packsc[:, 1, :], sqsc[:].rearrange("c (co x) -> c co x", x=4),
                axis=mybir.AxisListType.X, op=ALU.add)
            nc.sync.dma_start(arsc_in[:].rearrange("s (co c) -> c s co", c=128), packsc[:])
            nc.gpsimd.collective_compute(
                "AllGather", ALU.bypass,
                replica_groups=[list(range(NCORES))],
                ins=[arsc_in.opt()], outs=[arsc_out.opt()])
            nc.sync.dma_start(
                stgsc[:].rearrange("c r s co -> c (r s co)"),
                arsc_out[:].rearrange("r s (co c) -> c (r s co)", c=128))

            # BN scale/shift from raw (sum, sumsq) stats: writes scale_t/shift_t
            def bn_affine(st, g_row, b_row, scale_t, shift_t, t1, t2):
                nc.vector.tensor_scalar_mul(st[:], st[:], 1.0 / CNT)
                mean = st[:, 0, :]
                nc.vector.tensor_tensor(t1, mean, mean, op=ALU.mult)
                nc.vector.tensor_tensor(t2, st[:, 1, :], t1, op=ALU.subtract)
                nc.scalar.activation(t2, t2, AFT.Sqrt, bias=epsc[:])
                nc.vector.reciprocal(t1, t2)
                nc.vector.tensor_tensor(scale_t, g_row, t1, op=ALU.mult)
                nc.vector.tensor_tensor(t1, mean, scale_t, op=ALU.mult)
                nc.vector.tensor_tensor(shift_t, b_row, t1, op=ALU.subtract)

            # ---- BN1 scale/shift from global stats ----
            bn_affine(st1, gbv[:, 0, :], gbv[:, 1, :], scale1[:], shift1[:],
                      tmpa[:], tmpb[:])

            # ---- BN1 apply + ReLU -> padded conv2 input (interleave phases) ----
            for co in range(NCO):
                for iph, (p, q) in enumerate(PHASES):
                    dst = OP1[co][:, :, 1:33, 1:33] \
                        .rearrange("c b (i p2) (j q2) -> c b i p2 j q2", p2=2, q2=2)[:, :, :, p, :, q]
                    src = C1[co][:, iph * 512:(iph + 1) * 512] \
                        .rearrange("c (b h w) -> c b h w", b=2, h=16)
                    nc.scalar.activation(dst, src, AFT.Relu,
                                         bias=shift1[:, co:co + 1], scale=scale1[:, co:co + 1])

            # ---- BNsc scale/shift (emitted after BN1-apply so the Act-queue
            #      Sqrt waiting on AG#sc cannot delay BN1) ----
            nc.vector.tensor_reduce(
                stsc[:], stgsc[:].rearrange("c r s co -> c s co r"),
                axis=mybir.AxisListType.X, op=ALU.add)
            bn_affine(stsc, gbv[:, 4, :], gbv[:, 5, :], scalesc[:], shiftsc[:],
                      tmpa[:], tmpb[:])

            # ================= conv2 (3x3, pad 1, on OP1) =================
            for co in range(NCO):
                if co == 1:
                    # z = BNsc(csc), de-interleaved from phase-major CSC into
                    # raster order in the dead C1 tiles; hidden under conv2
                    for zco in range(NCO):
                        for iph, (p, q) in enumerate(PHASES):
                            nc.vector.tensor_scalar(
                                _phase_view(C1[zco][:], p, q),
                                CSC[zco][:, iph * 512:(iph + 1) * 512]
                                .rearrange("c (b h w) -> c b h w", b=2, h=16),
                                scalesc[:, zco:zco + 1], shiftsc[:, zco:zco + 1],
                                op0=ALU.mult, op1=ALU.add)
                pps = [ps.tile([128, 512], f32, name=f"c2ps{co}_{qq}", tag="psb")
                       for qq in range(4)]
                wt4 = [w2p.tile([128, 9, 128], bf16, name=f"c2w{ci}", tag="w2")
                       for ci in range(NCO)]
                for ci in range(NCO):
                    nc.sync.dma_start(wt4[ci][:], w2t_d[co, ci])
                # quarter-major accumulation: each quarter finishes (and its
                # stats drain starts) while later quarters are still on the PE
                n_acc = NCO * 9
                for qq in range(4):
                    k = 0
                    for ci in range(NCO):
                        for dh in (-1, 0, 1):
                            for dw in (-1, 0, 1):
                                t = (dh + 1) * 3 + (dw + 1)
                                rhs = OP1[ci][:, :, 1 + 8 * qq + dh:9 + 8 * qq + dh, 1 + dw:33 + dw]
                                nc.tensor.matmul(pps[qq][:], wt4[ci][:, t, :], rhs,
                                                 start=(k == 0), stop=(k == n_acc - 1))
                                k += 1
                    icol = co * 4 + qq
                    dst = C2[co][:].rearrange("c (b h w) -> c b h w", b=2, h=32)[:, :, 8 * qq:8 * qq + 8, :]
                    nc.vector.tensor_scalar(
                        dst, pps[qq][:].rearrange("c (b h w) -> c b h w", b=2, h=8),
                        0.0, 0.0, op0=ALU.add, op1=ALU.add,
                        accum_out=sums2[:, icol:icol + 1])
                    scr = scr_pool.tile([128, 512], f32, name="c2scr", tag="scr")
                    nc.scalar.activation(
                        scr[:], pps[qq][:], AFT.Square,
                        accum_out=sq2[:, icol:icol + 1])
                if co == 2:
                    # c2 stats for co 0..2 -> AllGather #A (hidden under co=3)
                    for row, src in ((0, sums2), (1, sq2)):
                        nc.vector.tensor_reduce(
                            packA[:, row, :],
                            src[:, :12].rearrange("c (co x) -> c co x", x=4),
                            axis=mybir.AxisListType.X, op=ALU.add)
                    nc.sync.dma_start(
                        arA_in[:].rearrange("s (co c) -> c s co", c=128), packA[:])
                    nc.gpsimd.collective_compute(
                        "AllGather", ALU.bypass,
                        replica_groups=[list(range(NCORES))],
                        ins=[arA_in.opt()], outs=[arA_out.opt()])

            # (emitted ahead of the packB DMA so it runs the moment AG#A lands)
            nc.sync.dma_start(
                stgA[:].rearrange("c r s co -> c (r s co)"),
                arA_out[:].rearrange("r s (co c) -> c (r s co)", c=128))

            # ---- c2 stats for co=3 -> AllGather #B (the only tail collective)
            for row, src in ((0, sums2), (1, sq2)):
                nc.vector.tensor_reduce(
                    packB[:, row, :],
                    src[:, 12:16].rearrange("c (co x) -> c co x", x=4),
                    axis=mybir.AxisListType.X, op=ALU.add)
            nc.sync.dma_start(
                arB_in[:].rearrange("s (co c) -> c s co", c=128),
                packB[:])
            nc.gpsimd.collective_compute(
                "AllGather", ALU.bypass,
                replica_groups=[list(range(NCORES))],
                ins=[arB_in.opt()], outs=[arB_out.opt()])

            # ---- BN2 for co 0..2 + their fuse/store, all under AllGather #B --
            nc.vector.tensor_reduce(
                stA[:], stgA[:].rearrange("c r s co -> c s co r"),
                axis=mybir.AxisListType.X, op=ALU.add)
            bn_affine(stA, gbv[:, 2, :3], gbv[:, 3, :3], scale2A[:], shift2A[:],
                      tmpa[:, :3], tmpb[:, :3])

            # final = relu(s2*c2 + t2 + z), z = BNsc(csc) in raster order in C1.
            # Chunked in 512-col pieces so no DVE/DMA op ever blocks the
            # critical stats path by more than ~0.7us
            def fuse_store(co, scale_t, shift_t):
                fin = FSTG[co % 2]
                for ck in range(4):
                    lo, hi = ck * 512, (ck + 1) * 512
                    nc.vector.scalar_tensor_tensor(
                        C2[co][:, lo:hi], C2[co][:, lo:hi], scale_t,
                        C1[co][:, lo:hi], op0=ALU.mult, op1=ALU.add)
                    nc.scalar.activation(fin[:, lo:hi], C2[co][:, lo:hi],
                                         AFT.Relu, bias=shift_t)
                    b = lo // 1024
                    nc.sync.dma_start(
                        out_d[b, co * 128:(co + 1) * 128]
                        .rearrange("c h w -> c (h w)")[:, (lo % 1024):(lo % 1024) + 512],
                        fin[:, lo:hi])

            for co in range(3):
                fuse_store(co, scale2A[:, co:co + 1], shift2A[:, co:co + 1])

            # ---- tail: co=3 stats land, BN2, fuse, store ----
            nc.sync.dma_start(
                stgB[:].rearrange("c r s -> c (r s)"),
                arB_out[:].rearrange("r s (co c) -> c (r s co)", c=128))
            nc.vector.tensor_reduce(
                stB[:, :, 0], stgB[:].rearrange("c r s -> c s r"),
                axis=mybir.AxisListType.X, op=ALU.add)
            bn_affine(stB, gbv[:, 2, 3:], gbv[:, 3, 3:], scale2B[:], shift2B[:],
                      tmpa[:, :1], tmpb[:, :1])
            fuse_store(3, scale2B[:], shift2B[:])

            for _f in _frees:
                _f()

    nc.compile()
    return nc


def _get_nc():
    if "nc" not in _CACHE:
        _CACHE["nc"] = _build_nc()
    return _CACHE["nc"]


def _regroup_w5(wt_full: np.ndarray) -> np.ndarray:
    """[5,5,CIN,COUT] -> [200,128,COUT] blocks in kernel consumption order."""
    blocks = np.empty((200, 128, COUT), dtype=np.float32)
    g = 0
    for (p, q, ci, ah, kh, kws) in _w5_groups():
        for (aw, kw) in kws:
            blocks[g] = wt_full[kh, kw, ci * 128:(ci + 1) * 128, :]
            g += 1
    assert g == 200
    return blocks


def _prep_inputs(x, w1, w2, wsc, g1, b1, g2, b2, gsc, bsc):
    xpad = np.zeros((B, CIN, 18, 18), dtype=np.float32)
    xpad[:, :, 1:17, 1:17] = x
    xpad = _to_bf16(xpad)
    w1g = _to_bf16(_regroup_w5(w1.transpose(2, 3, 1, 0)))
    wscg = _to_bf16(_regroup_w5(wsc.transpose(2, 3, 1, 0)))
    # [co, ci, c, t, m]: direct per-(co,ci) [128, 9, 128] blocks for conv2
    w2t = np.ascontiguousarray(w2.transpose(2, 3, 1, 0)).reshape(9, COUT, COUT)
    w2t = _to_bf16(np.ascontiguousarray(
        w2t.reshape(9, 4, 128, 4, 128).transpose(3, 1, 2, 0, 4)))
    gb = np.stack([g1, b1, g2, b2, gsc, bsc]).astype(np.float32)   # [6, 512]
    gbt = np.ascontiguousarray(gb.reshape(6, 4, 128).transpose(2, 0, 1))  # [128, 6, 4]
    return xpad, w1g, wscg, w2t, gbt


def kernel(x, w1, g1, b1, w2, g2, b2, wsc, gsc, bsc, _trace=False, **_kw):
    from concourse.bass_utils import run_bass_kernel_spmd

    x = np.asarray(x, dtype=np.float32)
    xpad, w1g, wscg, w2t, gbt = _prep_inputs(
        np.asarray(x), np.asarray(w1), np.asarray(w2), np.asarray(wsc),
        np.asarray(g1), np.asarray(b1), np.asarray(g2), np.asarray(b2),
        np.asarray(gsc), np.asarray(bsc))

    nc = _get_nc()
    in_maps = []
    for core in range(NCORES):
        in_maps.append({
            "xpad": xpad[core * B_LOC:(core + 1) * B_LOC],
            "w1g": w1g, "wscg": wscg, "w2t": w2t, "gb": gbt,
            "zpad": _to_bf16(np.zeros((2, 34, 34), dtype=np.float32)),
        })
    res = run_bass_kernel_spmd(nc, in_maps, list(range(NCORES)), trace=_trace)
    out = np.concatenate([res.results[i]["out"] for i in range(NCORES)], axis=0)
    if _trace:
        _CACHE["last_result"] = res
    return out



# revision 58
# speedup vs baseline: 1.0865x; 1.0865x over previous
"""Trainium2 Bass kernel for the Gudi UpProj block.

Reference computation (per image, NCHW):
    xu  = zero_stuff_2x(x)                    # [B,1024,32,32], nonzero only at even (h,w)
    c1  = conv5x5(xu, w1, pad=2);  out1 = relu(BN(c1))
    c2  = conv3x3(out1, w2, pad=1)
    csc = conv5x5(xu, wsc, pad=2)
    out = relu(BN(c2) + BN(csc))              # BN: training-mode batch stats over (N,H,W)

Strategy:
  * Data-parallel over batch: 16 images -> 2 per NeuronCore (8 cores).
  * Zero-stuffing exploited: a 5x5 conv on the zero-stuffed 32x32 grid decomposes
    into 4 parity phases, each a small conv (3x3 / 3x2 / 2x3 / 2x2) on the original
    16x16 grid -> 4x FLOP reduction.
  * All convs as tap-decomposed matmuls on the PE array in float32r
    (TF32-like: full PE rate at N>=256, ~1e-4 rel err). Weights / x are
    pre-rounded host-side (RNE to 11 mantissa bits), regrouped into the exact
    consumption order, and DMA'd directly in multi-tap batches.
  * BN batch stats (sum, sumsq per channel) need cross-core reduction:
    two small AllReduces (stats of c1; stats of c2+csc together).
"""

import numpy as np

NCORES = 8
B = 16
B_LOC = B // NCORES          # 2 images per core
CIN, COUT = 1024, 512
NCI, NCO = CIN // 128, COUT // 128   # 8, 4 partition tiles
H = 16                        # input spatial
OH = 32                       # output spatial
EPS = 1e-5
CNT = float(B * OH * OH)      # BN element count per channel = 16384
PHASES = [(0, 0), (0, 1), (1, 0), (1, 1)]

_CACHE = {}


def _to_bf16(a: np.ndarray) -> np.ndarray:
    """Round fp32 to bfloat16 (RNE) - matmul operand dtype on the PE."""
    import ml_dtypes
    return np.ascontiguousarray(a, dtype=np.float32).astype(ml_dtypes.bfloat16)


def _taps(p):
    """Taps of a parity phase along one dim: list of (input shift, 5-tap kernel idx)."""
    if p == 0:
        return [(-1, 0), (0, 2), (1, 4)]
    return [(0, 1), (1, 3)]


def _w5_groups():
    """Weight-block groups for the phase-decomposed 5x5 conv, in consumption
    order: one group per (phase, cin-tile, kernel-row) holding len(kws) blocks."""
    groups = []
    for (p, q) in PHASES:
        for ci in range(NCI):
            for (ah, kh) in _taps(p):
                groups.append((p, q, ci, ah, kh, _taps(q)))
    return groups


def _phase_view(ap2048, p, q):
    """[128, 2048] tile viewed as [128, b, i, j] at output positions (2i+p, 2j+q)."""
    v = ap2048.rearrange("c (b i p2 j q2) -> c b i p2 j q2", b=2, i=16, p2=2, j=16, q2=2)
    return v[:, :, :, p, :, q]


def _build_nc():
    import concourse.mybir as mybir
    import concourse.tile as tile
    from concourse import bacc

    f32 = mybir.dt.float32
    bf16 = mybir.dt.bfloat16
    ALU = mybir.AluOpType
    AFT = mybir.ActivationFunctionType

    nc = bacc.Bacc("TRN2", target_bir_lowering=False, debug=False)

    # ---- kernel I/O ----
    xpad_d = nc.dram_tensor("xpad", [B_LOC, CIN, 18, 18], bf16, kind="ExternalInput").ap()
    w1g_d = nc.dram_tensor("w1g", [200, 128, COUT], bf16, kind="ExternalInput").ap()
    wscg_d = nc.dram_tensor("wscg", [200, 128, COUT], bf16, kind="ExternalInput").ap()
    w2t_d = nc.dram_tensor("w2t", [NCO, NCO, 128, 4, 3, 128], bf16, kind="ExternalInput").ap()
    gb_d = nc.dram_tensor("gb", [128, 6, 4], f32, kind="ExternalInput").ap()
    zpad_d = nc.dram_tensor("zpad", [2, 34, 34], bf16, kind="ExternalInput").ap()
    out_d = nc.dram_tensor("out", [B_LOC, COUT, OH, OH], f32, kind="ExternalOutput").ap()

    with tile.TileContext(nc) as tc:
        # collective buffers (internal DRAM)
        _frees = []
        ar1_in, _f = tc.tile([2, COUT], f32, space="DRAM", name="ar1_in"); _frees.append(_f)
        ar1_out, _f = tc.tile([2, COUT], f32, space="DRAM", addr_space="Shared", name="ar1_out"); _frees.append(_f)
        arsc_in, _f = tc.tile([2, COUT], f32, space="DRAM", name="arsc_in"); _frees.append(_f)
        arsc_out, _f = tc.tile([NCORES, 2, COUT], f32, space="DRAM", addr_space="Shared", name="arsc_out"); _frees.append(_f)
        arA_in, _f = tc.tile([2, 384], f32, space="DRAM", name="arA_in"); _frees.append(_f)
        arA_out, _f = tc.tile([NCORES, 2, 384], f32, space="DRAM", addr_space="Shared", name="arA_out"); _frees.append(_f)
        arB_in, _f = tc.tile([2, 128], f32, space="DRAM", name="arB_in"); _frees.append(_f)
        arB_out, _f = tc.tile([NCORES, 2, 128], f32, space="DRAM", addr_space="Shared", name="arB_out"); _frees.append(_f)

        with tc.tile_pool(name="xp", bufs=1) as xp_pool, \
             tc.tile_pool(name="acts", bufs=1) as acts, \
             tc.tile_pool(name="op1", bufs=1) as op1_pool, \
             tc.tile_pool(name="wts", bufs=4) as wts, \
             tc.tile_pool(name="w2p", bufs=8) as w2p, \
             tc.tile_pool(name="scr", bufs=1) as scr_pool, \
             tc.tile_pool(name="small", bufs=1) as small, \
             tc.tile_pool(name="ps", bufs=8, space="PSUM") as ps:

            # ---- persistent SBUF tensors ----
            XP = [xp_pool.tile([128, 2, 18, 18], bf16, name=f"xp{i}", tag=f"xp{i}")
                  for i in range(NCI)]
            C1 = [acts.tile([128, 2048], f32, name=f"c1_{i}", tag=f"c1_{i}") for i in range(NCO)]
            CSC = [acts.tile([128, 2048], bf16, name=f"csc_{i}", tag=f"csc_{i}") for i in range(NCO)]
            C2 = [acts.tile([128, 2048], bf16, name=f"c2_{i}", tag=f"c2_{i}") for i in range(NCO)]
            OP1 = [op1_pool.tile([128, 2, 34, 34], bf16, name=f"op1_{i}", tag=f"op1_{i}")
                   for i in range(NCO)]
            # conv2 input in the width-Winograd F(2,3) domain: [c, mw, b, h, j]
            VW = [op1_pool.tile([128, 4, 2, 34, 16], bf16, name=f"vw{i}", tag=f"vw{i}")
                  for i in range(NCO)]

            # stat columns: sums/sumsqs per (tensor, co, phase-or-quarter)
            sums1 = small.tile([128, 16], f32, name="sums1")
            sq1 = small.tile([128, 16], f32, name="sq1")
            sums2 = small.tile([128, 16], f32, name="sums2")
            sq2 = small.tile([128, 16], f32, name="sq2")
            sumssc = small.tile([128, 16], f32, name="sumssc")
            sqsc = small.tile([128, 16], f32, name="sqsc")
            pack1 = small.tile([128, 2, 4], f32, name="pack1")
            st1 = small.tile([128, 2, 4], f32, name="st1")
            gbv = small.tile([128, 6, 4], f32, name="gbv")      # rows: g1,b1,g2,b2,gsc,bsc
            scale1 = small.tile([128, 4], f32, name="scale1")
            shift1 = small.tile([128, 4], f32, name="shift1")
            packsc = small.tile([128, 2, 4], f32, name="packsc")
            stgsc = small.tile([128, NCORES, 2, 4], f32, name="stgsc")
            stsc = small.tile([128, 2, 4], f32, name="stsc")
            scalesc = small.tile([128, 4], f32, name="scalesc")
            shiftsc = small.tile([128, 4], f32, name="shiftsc")
            packA = small.tile([128, 2, 3], f32, name="packA")
            stgA = small.tile([128, NCORES, 2, 3], f32, name="stgA")
            stA = small.tile([128, 2, 3], f32, name="stA")
            scale2A = small.tile([128, 3], f32, name="scale2A")
            shift2A = small.tile([128, 3], f32, name="shift2A")
            packB = small.tile([128, 2, 1], f32, name="packB")
            stgB = small.tile([128, NCORES, 2], f32, name="stgB")
            stB = small.tile([128, 2, 1], f32, name="stB")
            scale2B = small.tile([128, 1], f32, name="scale2B")
            shift2B = small.tile([128, 1], f32, name="shift2B")
            tmpa = small.tile([128, 4], f32, name="tmpa")
            tmpb = small.tile([128, 4], f32, name="tmpb")
            epsc = small.tile([128, 1], f32, name="epsc")
            FSTG = [small.tile([128, 2048], f32, name=f"fstg{i}") for i in range(2)]

            # ---- input DMAs (x first: the PE's first dependency) ----
            def emit_xp_dma(ci):
                nc.sync.dma_start(
                    XP[ci][:].rearrange("c b h w -> c b (h w)"),
                    xpad_d[:, ci * 128:(ci + 1) * 128].rearrange("b c h w -> c b (h w)"),
                )
            emit_xp_dma(0)
            nc.vector.memset(epsc[:], EPS)

            # ---- PE warmup: dummy matmuls on a memset tile while the first
            # x/weight DMAs are in flight, so the PE p-state is fully ramped
            # (>3us continuous busy) when real work arrives ----
            warm = small.tile([128, 512], bf16, name="warm")
            nc.vector.memset(warm[:], 0.0)
            wps = ps.tile([128, 512], f32, name="warmps", tag="psb")
            NWARM = 14
            for wi in range(NWARM):
                nc.tensor.matmul(wps[:, :256], warm[:, :128], warm[:, :256],
                                 start=(wi == 0), stop=(wi == NWARM - 1))

            # ---- helper: one 5x5-phase-decomposed conv (conv1 / convsc) ----
            def conv5(wg_d, dst, sums, sqs, wtag, prefetch_xp=False):
                gofs = 0
                for iph, (p, q) in enumerate(PHASES):
                    pps = [ps.tile([128, 512], f32, name=f"{wtag}ps{iph}_{co}", tag="psb")
                           for co in range(NCO)]
                    kws = _taps(q)
                    n_acc = NCI * len(_taps(p)) * len(kws)
                    k = 0
                    for ci in range(NCI):
                        for ti, (ah, kh) in enumerate(_taps(p)):
                            L = len(kws)
                            wt = wts.tile([128, 3, 512], bf16, name=f"{wtag}w", tag="w5")
                            if prefetch_xp and gofs == 0:
                                # per-tap DMAs so the very first matmul's
                                # weights land sooner
                                for kwi in range(L):
                                    nc.sync.dma_start(
                                        wt[:, kwi:kwi + 1, :],
                                        wg_d[kwi:kwi + 1].rearrange("l c m -> c l m"))
                            else:
                                nc.sync.dma_start(
                                    wt[:, :L, :],
                                    wg_d[gofs:gofs + L].rearrange("l c m -> c l m"))
                            gofs += L
                            # x-tile prefetch behind the first weight group so
                            # the first matmul's dependencies DMA first
                            if prefetch_xp and iph == 0 and ti == 0:
                                if ci == 0:
                                    emit_xp_dma(1)
                                if ci + 2 < NCI:
                                    emit_xp_dma(ci + 2)
                            for kwi, (aw, kw) in enumerate(kws):
                                rhs = XP[ci][:, :, 1 + ah:17 + ah, 1 + aw:17 + aw]
                                for co in range(NCO):
                                    nc.tensor.matmul(
                                        pps[co][:], wt[:, kwi, co * 128:(co + 1) * 128], rhs,
                                        start=(k == 0), stop=(k == n_acc - 1))
                                k += 1
                    for co in range(NCO):
                        icol = co * 4 + iph
                        nc.vector.tensor_scalar(
                            dst[co][:, iph * 512:(iph + 1) * 512], pps[co][:],
                            0.0, 0.0, op0=ALU.add, op1=ALU.add,
                            accum_out=sums[:, icol:icol + 1])
                        scr = scr_pool.tile([128, 512], f32, name=f"{wtag}scr", tag="scr")
                        nc.scalar.activation(
                            scr[:], pps[co][:], AFT.Square,
                            accum_out=sqs[:, icol:icol + 1])

            # ================= conv1 =================
            conv5(w1g_d, C1, sums1, sq1, "c1", prefetch_xp=True)

            # aux DMAs (needed from BN1-apply onward; emitted late to keep the
            # startup DMA path clear)
            nc.sync.dma_start(gbv[:], gb_d)
            for co in range(NCO):
                nc.sync.dma_start(OP1[co][:], zpad_d.unsqueeze(0).partition_broadcast(128))

            # ---- c1 stats -> AllReduce #1 (overlaps with convsc compute) ----
            nc.vector.tensor_reduce(
                pack1[:, 0, :], sums1[:].rearrange("c (co ph) -> c co ph", ph=4),
                axis=mybir.AxisListType.X, op=ALU.add)
            nc.vector.tensor_reduce(
                pack1[:, 1, :], sq1[:].rearrange("c (co ph) -> c co ph", ph=4),
                axis=mybir.AxisListType.X, op=ALU.add)
            nc.sync.dma_start(ar1_in[:].rearrange("s (co c) -> c s co", c=128), pack1[:])
            nc.gpsimd.collective_compute(
                "AllReduce", ALU.add,
                replica_groups=[list(range(NCORES))],
                ins=[ar1_in.opt()], outs=[ar1_out.opt()])
            nc.sync.dma_start(st1[:], ar1_out[:].rearrange("s (co c) -> c s co", c=128))

            # ================= convsc (independent of BN1) =================
            conv5(wscg_d, CSC, sumssc, sqsc, "sc")

            # conv2 transformed-weight prefetch: emitted here so the DMAs run
            # the moment convsc's weight stream drains
            W4 = {}

            def fetch_w4(co):
                W4[co] = [w2p.tile([128, 4, 3, 128], bf16, name=f"w2w{co}_{ci}",
                                   tag="w2") for ci in range(NCO)]
                for ci in range(NCO):
                    nc.sync.dma_start(W4[co][ci][:], w2t_d[co, ci])

            fetch_w4(0)
            fetch_w4(1)

            # ---- csc stats -> AllGather #sc (overlaps BN1-apply + conv2) ----
            nc.vector.tensor_reduce(
                packsc[:, 0, :], sumssc[:].rearrange("c (co x) -> c co x", x=4),
                axis=mybir.AxisListType.X, op=ALU.add)
            nc.vector.tensor_reduce(
                packsc[:, 1, :], sqsc[:].rearrange("c (co x) -> c co x", x=4),
                axis=mybir.AxisListType.X, op=ALU.add)
            nc.sync.dma_start(arsc_in[:].rearrange("s (co c) -> c s co", c=128), packsc[:])
            nc.gpsimd.collective_compute(
                "AllGather", ALU.bypass,
                replica_groups=[list(range(NCORES))],
                ins=[arsc_in.opt()], outs=[arsc_out.opt()])
            nc.sync.dma_start(
                stgsc[:].rearrange("c r s co -> c (r s co)"),
                arsc_out[:].rearrange("r s (co c) -> c (r s co)", c=128))

            # BN scale/shift from raw (sum, sumsq) stats: writes scale_t/shift_t
            def bn_affine(st, g_row, b_row, scale_t, shift_t, t1, t2):
                nc.vector.tensor_scalar_mul(st[:], st[:], 1.0 / CNT)
                mean = st[:, 0, :]
                nc.vector.tensor_tensor(t1, mean, mean, op=ALU.mult)
                nc.vector.tensor_tensor(t2, st[:, 1, :], t1, op=ALU.subtract)
                nc.scalar.activation(t2, t2, AFT.Sqrt, bias=epsc[:])
                nc.vector.reciprocal(t1, t2)
                nc.vector.tensor_tensor(scale_t, g_row, t1, op=ALU.mult)
                nc.vector.tensor_tensor(t1, mean, scale_t, op=ALU.mult)
                nc.vector.tensor_tensor(shift_t, b_row, t1, op=ALU.subtract)

            # ---- BN1 scale/shift from global stats ----
            bn_affine(st1, gbv[:, 0, :], gbv[:, 1, :], scale1[:], shift1[:],
                      tmpa[:], tmpb[:])

            # ---- BN1 apply + ReLU -> padded conv2 input (interleave phases) ----
            for co in range(NCO):
                for iph, (p, q) in enumerate(PHASES):
                    dst = OP1[co][:, :, 1:33, 1:33] \
                        .rearrange("c b (i p2) (j q2) -> c b i p2 j q2", p2=2, q2=2)[:, :, :, p, :, q]
                    src = C1[co][:, iph * 512:(iph + 1) * 512] \
                        .rearrange("c (b h w) -> c b h w", b=2, h=16)
                    nc.scalar.activation(dst, src, AFT.Relu,
                                         bias=shift1[:, co:co + 1], scale=scale1[:, co:co + 1])

            # ---- BNsc scale/shift (emitted after BN1-apply so the Act-queue
            #      Sqrt waiting on AG#sc cannot delay BN1) ----
            nc.vector.tensor_reduce(
                stsc[:], stgsc[:].rearrange("c r s co -> c s co r"),
                axis=mybir.AxisListType.X, op=ALU.add)
            bn_affine(stsc, gbv[:, 4, :], gbv[:, 5, :], scalesc[:], shiftsc[:],
                      tmpa[:], tmpb[:])

            # ---- conv2 input transform (width-Winograd F(2,3)): built on the
            # otherwise-idle DVE as soon as each OP1 tile lands ----
            for ci in range(NCO):
                d = [OP1[ci][:, :, :, v:min(v + 32, 34):2] for v in range(4)]
                nc.vector.tensor_tensor(VW[ci][:, 0], d[0], d[2], op=ALU.subtract)
                nc.vector.tensor_tensor(VW[ci][:, 1], d[1], d[2], op=ALU.add)
                nc.vector.tensor_tensor(VW[ci][:, 2], d[2], d[1], op=ALU.subtract)
                nc.vector.tensor_tensor(VW[ci][:, 3], d[1], d[3], op=ALU.subtract)

            # ================= conv2 (3x3, width-Winograd, on VW) ==========
            # PE computes M[mw][co, b, h, j] = sum_{kh,ci} Wt[kh,mw]^T V[mw];
            # DVE combines y[2j] = M0+M1+M2, y[2j+1] = M1-M2-M3 into C2.
            for co in range(NCO):
                if co + 2 < NCO:
                    fetch_w4(co + 2)
                if co == 1:
                    # z = BNsc(csc), de-interleaved from phase-major CSC into
                    # raster order in the dead C1 tiles; hidden under conv2
                    for zco in range(NCO):
                        for iph, (p, q) in enumerate(PHASES):
                            nc.vector.tensor_scalar(
                                _phase_view(C1[zco][:], p, q),
                                CSC[zco][:, iph * 512:(iph + 1) * 512]
                                .rearrange("c (b h w) -> c b h w", b=2, h=16),
                                scalesc[:, zco:zco + 1], shiftsc[:, zco:zco + 1],
                                op0=ALU.mult, op1=ALU.add)
                wt4 = W4.pop(co)
                for hh in range(2):
                    pps = [ps.tile([128, 512], f32, name=f"c2ps{co}_{mw}", tag="psb")
                           for mw in range(4)]
                    for ci in range(NCO):
                        for mw in range(4):
                            for kh in range(3):
                                rhs = VW[ci][:, mw, :, 16 * hh + kh:16 * hh + kh + 16, :]
                                nc.tensor.matmul(
                                    pps[mw][:], wt4[ci][:, mw, kh, :], rhs,
                                    start=(ci == 0 and kh == 0),
                                    stop=(ci == NCO - 1 and kh == 2))
                    # inverse transform into C2 raster rows [16*hh, 16*hh+16)
                    m = [pps[i][:].rearrange("c (b h j) -> c b h j", b=2, h=16)
                         for i in range(4)]
                    cv = C2[co][:].rearrange("c (b h j q) -> c b h j q", h=32, j=16, q=2)
                    cv = cv[:, :, 16 * hh:16 * hh + 16]
                    # ISA: two DVE sources cannot both be PSUM -> stage M1 in
                    # SBUF, then each combine op reads at most one PSUM bank
                    tmA = scr_pool.tile([128, 512], f32, name="tmA", tag="scr")
                    tmB = scr_pool.tile([128, 512], f32, name="tmB", tag="scr2")
                    tmC = scr_pool.tile([128, 512], f32, name="tmC", tag="scr3")
                    tA = tmA[:].rearrange("c (b h j) -> c b h j", b=2, h=16)
                    tB = tmB[:].rearrange("c (b h j) -> c b h j", b=2, h=16)
                    tC = tmC[:].rearrange("c (b h j) -> c b h j", b=2, h=16)
                    nc.vector.tensor_scalar(tA, m[1], 0.0, None, op0=ALU.add)
                    nc.vector.tensor_tensor(tC, tA, m[0], op=ALU.add)
                    nc.vector.tensor_tensor(cv[:, :, :, :, 0], tC, m[2], op=ALU.add)
                    nc.vector.tensor_tensor(tB, tA, m[2], op=ALU.subtract)
                    nc.vector.tensor_tensor(cv[:, :, :, :, 1], tB, m[3], op=ALU.subtract)
                    # stats for the two images of this (co, hh) row block
                    for b in range(2):
                        icol = co * 4 + hh * 2 + b
                        sl = C2[co][:, b * 1024 + hh * 512:b * 1024 + hh * 512 + 512]
                        nc.vector.tensor_scalar(
                            sl, sl, 0.0, 0.0, op0=ALU.add, op1=ALU.add,
                            accum_out=sums2[:, icol:icol + 1])
                        scr = scr_pool.tile([128, 512], f32, name="c2scr", tag="scr")
                        nc.scalar.activation(
                            scr[:], sl, AFT.Square,
                            accum_out=sq2[:, icol:icol + 1])
                if co == 2:
                    # c2 stats for co 0..2 -> AllGather #A (hidden under co=3)
                    for row, src in ((0, sums2), (1, sq2)):
                        nc.vector.tensor_reduce(
                            packA[:, row, :],
                            src[:, :12].rearrange("c (co x) -> c co x", x=4),
                            axis=mybir.AxisListType.X, op=ALU.add)
                    nc.sync.dma_start(
                        arA_in[:].rearrange("s (co c) -> c s co", c=128), packA[:])
                    nc.gpsimd.collective_compute(
                        "AllGather", ALU.bypass,
                        replica_groups=[list(range(NCORES))],
                        ins=[arA_in.opt()], outs=[arA_out.opt()])

            # (emitted ahead of the packB DMA so it runs the moment AG#A lands)
            nc.sync.dma_start(
                stgA[:].rearrange("c r s co -> c (r s co)"),
                arA_out[:].rearrange("r s (co c) -> c (r s co)", c=128))

            # ---- c2 stats for co=3 -> AllGather #B (the only tail collective)
            for row, src in ((0, sums2), (1, sq2)):
                nc.vector.tensor_reduce(
                    packB[:, row, :],
                    src[:, 12:16].rearrange("c (co x) -> c co x", x=4),
                    axis=mybir.AxisListType.X, op=ALU.add)
            nc.sync.dma_start(
                arB_in[:].rearrange("s (co c) -> c s co", c=128),
                packB[:])
            nc.gpsimd.collective_compute(
                "AllGather", ALU.bypass,
                replica_groups=[list(range(NCORES))],
                ins=[arB_in.opt()], outs=[arB_out.opt()])

            # ---- BN2 for co 0..2 + their fuse/store, all under AllGather #B --
            nc.vector.tensor_reduce(
                stA[:], stgA[:].rearrange("c r s co -> c s co r"),
                axis=mybir.AxisListType.X, op=ALU.add)
            bn_affine(stA, gbv[:, 2, :3], gbv[:, 3, :3], scale2A[:], shift2A[:],
                      tmpa[:, :3], tmpb[:, :3])

            # final = relu(s2*c2 + t2 + z), z = BNsc(csc) in raster order in C1.
            # Chunked in 512-col pieces so no DVE/DMA op ever blocks the
            # critical stats path by more than ~0.7us
            def fuse_store(co, scale_t, shift_t):
                fin = FSTG[co % 2]
                for ck in range(4):
                    lo, hi = ck * 512, (ck + 1) * 512
                    nc.vector.scalar_tensor_tensor(
                        C2[co][:, lo:hi], C2[co][:, lo:hi], scale_t,
                        C1[co][:, lo:hi], op0=ALU.mult, op1=ALU.add)
                    nc.scalar.activation(fin[:, lo:hi], C2[co][:, lo:hi],
                                         AFT.Relu, bias=shift_t)
                    b = lo // 1024
                    nc.sync.dma_start(
                        out_d[b, co * 128:(co + 1) * 128]
                        .rearrange("c h w -> c (h w)")[:, (lo % 1024):(lo % 1024) + 512],
                        fin[:, lo:hi])

            for co in range(3):
                fuse_store(co, scale2A[:, co:co + 1], shift2A[:, co:co + 1])

            # ---- tail: co=3 stats land, BN2, fuse, store ----
            nc.sync.dma_start(
                stgB[:].rearrange("c r s -> c (r s)"),
                arB_out[:].rearrange("r s (co c) -> c (r s co)", c=128))
            nc.vector.tensor_reduce(
                stB[:, :, 0], stgB[:].rearrange("c r s -> c s r"),
                axis=mybir.AxisListType.X, op=ALU.add)
            bn_affine(stB, gbv[:, 2, 3:], gbv[:, 3, 3:], scale2B[:], shift2B[:],
                      tmpa[:, :1], tmpb[:, :1])
            fuse_store(3, scale2B[:], shift2B[:])

            for _f in _frees:
                _f()

    nc.compile()
    return nc


def _get_nc():
    if "nc" not in _CACHE:
        _CACHE["nc"] = _build_nc()
    return _CACHE["nc"]


def _regroup_w5(wt_full: np.ndarray) -> np.ndarray:
    """[5,5,CIN,COUT] -> [200,128,COUT] blocks in kernel consumption order."""
    blocks = np.empty((200, 128, COUT), dtype=np.float32)
    g = 0
    for (p, q, ci, ah, kh, kws) in _w5_groups():
        for (aw, kw) in kws:
            blocks[g] = wt_full[kh, kw, ci * 128:(ci + 1) * 128, :]
            g += 1
    assert g == 200
    return blocks


def _prep_inputs(x, w1, w2, wsc, g1, b1, g2, b2, gsc, bsc):
    xpad = np.zeros((B, CIN, 18, 18), dtype=np.float32)
    xpad[:, :, 1:17, 1:17] = x
    xpad = _to_bf16(xpad)
    w1g = _to_bf16(_regroup_w5(w1.transpose(2, 3, 1, 0)))
    wscg = _to_bf16(_regroup_w5(wsc.transpose(2, 3, 1, 0)))
    # width-Winograd F(2,3) transformed conv2 weights: [co, ci, c, mw, kh, m]
    G = np.array([[1, 0, 0], [.5, .5, .5], [.5, -.5, .5], [0, 0, 1]], np.float32)
    wt = np.einsum('vw,oihw->vhio', G, np.asarray(w2, dtype=np.float32))
    w2t = _to_bf16(np.ascontiguousarray(
        wt.reshape(4, 3, 4, 128, 4, 128).transpose(4, 2, 3, 0, 1, 5)))
    gb = np.stack([g1, b1, g2, b2, gsc, bsc]).astype(np.float32)   # [6, 512]
    gbt = np.ascontiguousarray(gb.reshape(6, 4, 128).transpose(2, 0, 1))  # [128, 6, 4]
    return xpad, w1g, wscg, w2t, gbt


def kernel(x, w1, g1, b1, w2, g2, b2, wsc, gsc, bsc, _trace=False, **_kw):
    from concourse.bass_utils import run_bass_kernel_spmd

    x = np.asarray(x, dtype=np.float32)
    xpad, w1g, wscg, w2t, gbt = _prep_inputs(
        np.asarray(x), np.asarray(w1), np.asarray(w2), np.asarray(wsc),
        np.asarray(g1), np.asarray(b1), np.asarray(g2), np.asarray(b2),
        np.asarray(gsc), np.asarray(bsc))

    nc = _get_nc()
    in_maps = []
    for core in range(NCORES):
        in_maps.append({
            "xpad": xpad[core * B_LOC:(core + 1) * B_LOC],
            "w1g": w1g, "wscg": wscg, "w2t": w2t, "gb": gbt,
            "zpad": _to_bf16(np.zeros((2, 34, 34), dtype=np.float32)),
        })
    res = run_bass_kernel_spmd(nc, in_maps, list(range(NCORES)), trace=_trace)
    out = np.concatenate([res.results[i]["out"] for i in range(NCORES)], axis=0)
    if _trace:
        _CACHE["last_result"] = res
    return out

